# revision 27
# baseline (speedup 1.0000x reference)
"""Multi-head attention forward on 8 Trainium2 NeuronCores (Bass/Tile).

Problem: x[4, 2048, 768] -> qkv proj (w_qkv[2304, 768]) -> 12-head attention
(softmax((q k^T) * 768^-0.5)) -> out proj (w_out[768, 768]).

Sharding: core c handles batch b = c//2 and a group of 6 heads g = c%2
(tensor parallel over heads within a batch pair). Each core computes a
partial output (its heads' contribution through the row-sliced out
projection, transposed: [768, 2048] bf16); the host sums the two partials
per batch, transposes back and adds b_out.

Device-side layout notes (everything transposed so the contraction dim sits
on SBUF partitions):
  xT   [768, 2048]  pre-transposed on the host (no on-chip transposes)
  qkvT [feat, 2048] = wT.T @ xT via bf16 matmuls
  q/k for a head pair share one [128, 2, N] tile: head A rows 0:64,
  head B rows 64:128; scores use K=64 matmuls with partition offsets
  (row_grp q0/q64) so no zero padding or memsets are needed.
  scoresT[keys, q] = kT_tile.T @ qT  (so attn@v needs no transpose)
  softmax without max-subtraction (scores are O(1); exp is safe in fp32);
  v_aug carries ones in cols 64:128 of each head block so PSUM rows 64:128
  of the attn@v accumulation hold the softmax denominator; the reciprocal
  is a fast custom-DVE op, broadcast across partitions by a tiny PE
  matmul, and the divide is a DVE multiply.
"""

import sys

import ml_dtypes
import numpy as np

_bf16 = ml_dtypes.bfloat16

if "/opt/trn_rl_repo" not in sys.path:
    sys.path.insert(0, "/opt/trn_rl_repo")

B = 4
N = 2048
DIM = 768
HEADS = 12
DHEAD = 64
SCALE = DIM ** (-0.5)
NCORES = 8
HPC = 6  # heads per core
FEAT = HPC * DHEAD  # 384 per-core attention features

_PROGRAM = None  # cached compiled bass program


def _source_salt():
    import hashlib
    import inspect

    src = inspect.getsource(_build_program)
    return int(hashlib.sha256(src.encode()).hexdigest(), 16) % 509 + 3


_SALT = None  # set lazily in _build_program


def _build_program(debug_taps=False):
    global _SALT
    _SALT = _source_salt() + (251 if debug_taps else 0)
    from contextlib import ExitStack

    import concourse.bass as bass
    import concourse.tile as tile
    from concourse import bacc, mybir
    from concourse.masks import make_identity

    f32 = mybir.dt.float32
    f32r = mybir.dt.float32r
    bf16 = mybir.dt.bfloat16
    Alu = mybir.AluOpType
    ActF = mybir.ActivationFunctionType

    nc = bacc.Bacc("TRN2", target_bir_lowering=False, debug=False)

    # The neuron NEFF cache keys on the HLO signature but not on the bass
    # program embedded in the custom-call config; encode a hash of this
    # builder's source into an input shape so program edits always miss.
    salt_in = nc.dram_tensor("salt", [1, _SALT], f32, kind="ExternalInput")

    xT_in = nc.dram_tensor("xT", [DIM, N], bf16, kind="ExternalInput")
    wqkvT = nc.dram_tensor("wqkvT", [DIM, 3 * FEAT], bf16, kind="ExternalInput")
    bqkv = nc.dram_tensor("bqkv", [128, 9], f32, kind="ExternalInput")
    woutT = nc.dram_tensor("woutT", [FEAT, DIM], bf16, kind="ExternalInput")
    out_T = nc.dram_tensor("outT", [DIM, N], bf16, kind="ExternalOutput")
    if debug_taps:
        dbg_ao = nc.dram_tensor("dbg_ao", [128, 3 * N], bf16, kind="ExternalOutput")
        dbg_den = nc.dram_tensor("dbg_den", [24, 512], f32, kind="ExternalOutput")
        dbg_qk = nc.dram_tensor("dbg_qk", [128, 4 * N], bf16, kind="ExternalOutput")

    NT = N // 128  # 16 key tiles
    KC = DIM // 128  # 6 contraction chunks for dim
    NSPAN = N // 512  # 4 moving spans

    with tile.TileContext(nc) as tc, ExitStack() as ctx:
        const = ctx.enter_context(tc.tile_pool(name="const", bufs=1))
        identity_bf = const.tile([128, 128], bf16)
        make_identity(nc, identity_bf)
        salt_sb = const.tile([1, _SALT], f32)
        nc.gpsimd.dma_start(salt_sb[:, :], salt_in[:, :])
        ones_f32 = const.tile([128, 1], f32)
        nc.vector.memset(ones_f32[:, :], 1.0)
        ones_bc = const.tile([65, 64], f32r)
        nc.vector.tensor_copy(
            out=ones_bc[:, :], in_=ones_f32[0:65, :].to_broadcast((65, 64))
        )
        bias_sb = const.tile([128, 9], f32)

        big = ctx.enter_context(tc.tile_pool(name="big", bufs=1))
        xT = big.tile([128, KC, N], bf16)
        w_all = big.tile([128, KC, 3 * FEAT], bf16)
        wout_sb = big.tile([128, 3, DIM], bf16)
        # [p, parity, q/k, tok]: head A rows 0:64, head B rows 64:128
        qk2 = big.tile([128, 2, 2, N], bf16)
        vT2 = big.tile([128, 2, N], bf16)
        # [p(keys), parity, kc, 2*(64 vdims | ones col 64 | zeros)]
        vaug2 = big.tile([128, 2, NT, 256], bf16)
        attn_outT = big.tile([128, 3, N], bf16)

        # one-time init: zeros + a single ones column per head block (zeros
        # keep the attn@v padding MACs from toggling — power/throttle)
        nc.vector.memset(vaug2[:, :, :, :], 0.0)
        for par in range(2):
            nc.vector.tensor_copy(
                out=vaug2[:, par, :, :].rearrange("p k (t c) -> p k t c", t=2)[
                    :, :, :, 64:65
                ],
                in_=ones_f32[:, :].to_broadcast((128, NT, 2, 1)),
            )

        # startup DMAs: interleave x/w per contraction chunk so the first
        # qkv chains unblock progressively
        for j in range(KC):
            nc.gpsimd.dma_start(xT[:, j, 0:1024], xT_in[j * 128 : (j + 1) * 128, 0:1024])
            nc.sync.dma_start(w_all[:, j, 0:768], wqkvT[j * 128 : (j + 1) * 128, 0:768])
        for j in range(KC):
            nc.gpsimd.dma_start(
                xT[:, j, 1024:2048], xT_in[j * 128 : (j + 1) * 128, 1024:2048]
            )
            nc.sync.dma_start(
                w_all[:, j, 768:1152], wqkvT[j * 128 : (j + 1) * 128, 768:1152]
            )
        for c in range(3):
            nc.sync.dma_start(wout_sb[:, c, :], woutT[c * 128 : (c + 1) * 128, :])
        nc.gpsimd.dma_start(bias_sb[:, :], bqkv[:, :])

        # PSUM: spool 3x[128,2,512] (6 banks) + opool 2x[128,512] (2 banks)
        spool = ctx.enter_context(tc.tile_pool(name="spsum", bufs=3, space="PSUM"))
        opool = ctx.enter_context(tc.tile_pool(name="opsum", bufs=2, space="PSUM"))

        exp_pool = ctx.enter_context(tc.tile_pool(name="expT", bufs=6))
        rcp_pool = ctx.enter_context(tc.tile_pool(name="rcp", bufs=2))
        sbo_pool = ctx.enter_context(tc.tile_pool(name="sbo", bufs=3))
        hst_pool = ctx.enter_context(tc.tile_pool(name="hstage", bufs=2))
        ost_pool = ctx.enter_context(tc.tile_pool(name="ostage", bufs=3))

        def emit_qkv_chain(par, idx, m, sp2):
            """One (m, span-pair): 12 matmuls + biased copyback."""
            ps = spool.tile([128, 2, 512], f32, tag="s", name="ps_qkv")
            for u in range(2):
                span = 2 * sp2 + u
                for j in range(KC):
                    nc.tensor.matmul(
                        ps[:, u, :],
                        w_all[:, j, m * 128 : (m + 1) * 128],
                        xT[:, j, span * 512 : (span + 1) * 512],
                        start=(j == 0),
                        stop=(j == KC - 1),
                    )
            cols = slice(sp2 * 1024, (sp2 + 1) * 1024)
            dst = qk2[:, par, idx, cols] if idx < 2 else vT2[:, par, cols]
            nc.vector.tensor_scalar(
                dst.rearrange("p (a b) -> p a b", a=2),
                ps[:, :, :],
                bias_sb[:, m : m + 1],
                None,
                Alu.add,
            )

        def emit_vtrans(par, kc0, kcn):
            for kc in range(kc0, kcn):
                tp = spool.tile([128, 128], bf16, tag="s", name="tp_v")
                nc.tensor.transpose(
                    tp[:, :], vT2[:, par, kc * 128 : (kc + 1) * 128], identity_bf[:, :]
                )
                nc.vector.tensor_copy(
                    out=vaug2[:, par, kc, :].rearrange("p (t c) -> p t c", t=2)[
                        :, :, 0:64
                    ],
                    in_=tp[:, :].rearrange("p (t c) -> p t c", t=2),
                )

        def make_pair_units(hp):
            """Return the PE filler closures building head pair hp's inputs."""
            par = hp % 2
            units = []
            order = [
                (0, hp, 0),
                (1, 3 + hp, 0),
                (2, 6 + hp, 0),
                (0, hp, 1),
                (1, 3 + hp, 1),
                (2, 6 + hp, 1),
            ]
            for idx, m, sp2 in order:
                units.append(lambda i=idx, mm=m, s=sp2: emit_qkv_chain(par, i, mm, s))
            for kc0 in (0, 4, 8, 12):
                units.append(lambda k=kc0: emit_vtrans(par, k, k + 4))
            return units

        def emit_outproj(m, span):
            """One out-proj tile [128, 512]: 3 matmuls + copyback + DMA."""
            ps = spool.tile([128, 2, 512], f32, tag="s", name="ps_op")
            for c in range(3):
                nc.tensor.matmul(
                    ps[:, 0, :],
                    wout_sb[:, c, m * 128 : (m + 1) * 128],
                    attn_outT[:, c, span * 512 : (span + 1) * 512],
                    start=(c == 0),
                    stop=(c == 2),
                )
            ostage = ost_pool.tile([128, 512], bf16, name="ostage", tag="ostage")
            nc.vector.tensor_copy(out=ostage[:, :], in_=ps[:, 0, :])
            eng = nc.gpsimd if m % 2 == 0 else nc.sync
            eng.dma_start(
                out_T[m * 128 : (m + 1) * 128, span * 512 : (span + 1) * 512],
                ostage[:, :],
            )

        # ---- phase 1: head pair 0 inputs ----
        for u in make_pair_units(0):
            u()

        # ---- attention per head pair ----
        # The activity governor clamps the PE to half rate when its windowed
        # duty stays near 1.0; attention alone is ACT(exp)-paced at ~0.8 PE
        # duty, so the next pair's qkv/vtrans units run as a dense burst at
        # the pair boundary instead of as in-attention filler.
        for hp in range(3):
            par = hp % 2
            filler = []  # hp==2: outproj units appended by normalize
            half_ctr = 0
            pending = [None]  # deferred normalize closure

            def flush_pending():
                if pending[0] is not None:
                    pending[0]()
                    pending[0] = None

            for j in range(2):
                rows = slice(0, 64) if j == 0 else slice(64, 128)
                for span in range(NSPAN):
                    po = opool.tile([128, 512], f32, tag="o", name="po")
                    ets = []
                    for half in range(8):
                        ps = spool.tile([128, 2, 512], f32, tag="s", name="ps_s")
                        for u in range(2):
                            kc = 2 * half + u
                            nc.tensor.matmul(
                                ps[:, u, :],
                                qk2[rows, par, 1, kc * 128 : (kc + 1) * 128],
                                qk2[rows, par, 0, span * 512 : (span + 1) * 512],
                                start=True,
                                stop=True,
                            )
                        et = exp_pool.tile([128, 2, 512], bf16)
                        nc.scalar.activation(
                            et[:, :, :], ps[:, :, :], ActF.Exp, scale=float(SCALE)
                        )
                        ets.append(et)
                        if half == 4:
                            # deferred: by now the previous block's reciprocal
                            # (3.4us on DVE) has certainly finished
                            flush_pending()
                        if half >= 1:
                            pet = ets[half - 1]
                            for u in range(2):
                                kc = 2 * (half - 1) + u
                                nc.tensor.matmul(
                                    po[:, :],
                                    vaug2[:, par, kc, j * 128 : (j + 1) * 128],
                                    pet[:, u, :],
                                    start=(kc == 0),
                                    stop=False,
                                )
                        half_ctr += 1
                        if filler:
                            filler.pop(0)()
                    pet = ets[7]
                    for u in range(2):
                        kc = 14 + u
                        nc.tensor.matmul(
                            po[:, :],
                            vaug2[:, par, kc, j * 128 : (j + 1) * 128],
                            pet[:, u, :],
                            start=False,
                            stop=(kc == 15),
                        )
                    # denominator recip + dims+denom PSUM->SBUF copy, both on
                    # DVE and issued immediately so the deferred broadcast
                    # matmul never waits (ACT stays the attention pacer)
                    rs = rcp_pool.tile([65, 512], f32r, name="rs", tag="rs")
                    with nc.allow_low_precision(reason="f32r recip"):
                        nc.vector.reciprocal(rs[64:65, :], po[64:65, :])
                    sb_o = sbo_pool.tile([65, 512], f32, name="sb_o")
                    nc.vector.tensor_copy(out=sb_o[:, :], in_=po[0:65, :])
                    if debug_taps:
                        bi = hp * 8 + j * 4 + span
                        nc.sync.dma_start(dbg_den[bi : bi + 1, :], sb_o[64:65, :])

                    def normalize(j=j, span=span, po=po, hp=hp, rs=rs, sb_o=sb_o):
                        # PE-broadcast of the recip back into po's own bank
                        # (WAR-ordered after the ACT copy), then divide.
                        nc.tensor.matmul(
                            po[0:64, :],
                            ones_bc[64:65, :],
                            rs[64:65, :],
                            start=True,
                            stop=True,
                        )
                        if j == 0:
                            ddst = attn_outT[0:64, hp, span * 512 : (span + 1) * 512]
                        else:
                            ddst = hst_pool.tile(
                                [64, 512], bf16, name="hstage", tag="hstage"
                            )
                        nc.vector.tensor_tensor(
                            out=ddst,
                            in0=sb_o[0:64, :],
                            in1=po[0:64, :],
                            op=Alu.mult,
                        )
                        if j == 1:
                            nc.gpsimd.dma_start(
                                attn_outT[64:128, hp, span * 512 : (span + 1) * 512],
                                ddst[:, :],
                            )
                            if hp == 2:
                                for m in range(DIM // 128):
                                    filler.append(
                                        lambda mm=m, s=span: emit_outproj(mm, s)
                                    )

                    pending[0] = normalize
            # pair-boundary burst: build the next pair's inputs; flush the
            # last block's normalize after the first burst unit so its
            # reciprocal has finished
            burst = make_pair_units(hp + 1) if hp < 2 else []
            if burst:
                burst[0]()
                flush_pending()
                for u in burst[1:]:
                    u()
            else:
                flush_pending()
            while filler:
                filler.pop(0)()

        if debug_taps:
            nc.gpsimd.dma_start(
                dbg_ao[:, :], attn_outT[:, :, :].rearrange("p a b -> p (a b)")
            )
            nc.gpsimd.dma_start(
                dbg_qk[:, 0 : 2 * N], qk2[:, 0, :, :].rearrange("p a b -> p (a b)")
            )
            nc.gpsimd.dma_start(
                dbg_qk[:, 2 * N : 4 * N], qk2[:, 1, :, :].rearrange("p a b -> p (a b)")
            )

    nc.compile()
    return nc


def _get_program():
    global _PROGRAM
    if _PROGRAM is None:
        _PROGRAM = _build_program()
    return _PROGRAM


def make_core_inputs(x, w_qkv, b_qkv, w_out):
    """Host-side shard: per-core input dicts for cores 0..7."""
    x = np.asarray(x, dtype=np.float32)
    w_qkv = np.asarray(w_qkv, dtype=np.float32)
    b_qkv = np.asarray(b_qkv, dtype=np.float32)
    w_out = np.asarray(w_out, dtype=np.float32)

    per_group = []
    for g in range(2):
        rows = np.concatenate(
            [
                w_qkv[qkv * DIM + g * FEAT : qkv * DIM + (g + 1) * FEAT]
                for qkv in range(3)
            ],
            axis=0,
        )  # [1152, 768]
        wqkvT_g = np.ascontiguousarray(rows.T).astype(_bf16)  # [768, 1152]
        b_rows = np.concatenate(
            [
                b_qkv[qkv * DIM + g * FEAT : qkv * DIM + (g + 1) * FEAT]
                for qkv in range(3)
            ],
            axis=0,
        )  # [1152]
        bias_g = np.ascontiguousarray(b_rows.reshape(9, 128).T)  # [128, 9]
        woutT_g = np.ascontiguousarray(w_out[:, g * FEAT : (g + 1) * FEAT].T).astype(
            _bf16
        )  # [384, 768]
        per_group.append((wqkvT_g, bias_g, woutT_g))

    xT_bf = [np.ascontiguousarray(x[b].T).astype(_bf16) for b in range(B)]
    salt = np.zeros((1, _SALT if _SALT is not None else _source_salt()), np.float32)
    in_maps = []
    for c in range(NCORES):
        b, g = c // 2, c % 2
        wqkvT_g, bias_g, woutT_g = per_group[g]
        in_maps.append(
            {
                "salt": salt,
                "xT": xT_bf[b],
                "wqkvT": wqkvT_g,
                "bqkv": bias_g,
                "woutT": woutT_g,
            }
        )
    return in_maps


def assemble_output(results, b_out):
    """Host-side unshard: sum partials per batch pair, transpose, add bias."""
    b_out = np.asarray(b_out, dtype=np.float32)
    out = np.empty((B, N, DIM), dtype=np.float32)
    for b in range(B):
        pT = results[2 * b]["outT"].astype(np.float32) + results[2 * b + 1][
            "outT"
        ].astype(np.float32)  # [768, 2048]
        out[b] = pT.T + b_out[None, :]
    return out


def kernel(x, w_qkv, b_qkv, w_out, b_out):
    from concourse.bass_utils import run_bass_kernel_spmd

    nc = _get_program()
    in_maps = make_core_inputs(x, w_qkv, b_qkv, w_out)
    res = run_bass_kernel_spmd(nc, in_maps, list(range(NCORES)))
    return assemble_output(res.results, b_out)


# revision 28
# speedup vs baseline: 1.2465x; 1.2465x over previous
"""Multi-head attention forward on 8 Trainium2 NeuronCores (Bass/Tile).

Problem: x[4, 2048, 768] -> qkv proj (w_qkv[2304, 768]) -> 12-head attention
(softmax((q k^T) * 768^-0.5)) -> out proj (w_out[768, 768]).

Sharding: core c handles batch b = c//2 and a group of 6 heads g = c%2
(tensor parallel over heads within a batch pair). Each core computes a
partial output (its heads' contribution through the row-sliced out
projection, transposed: [768, 2048]); the host sums the two partials per
batch, transposes back and adds b_out.

Device-side layout notes (everything transposed so the contraction dim sits
on SBUF partitions):
  xT   [768, 2048]  built on-chip via PE transposes of x tiles
  qkvT [feat, 2048] = wT.T @ xT via fp32r matmuls (full-rate fp32)
  scoresT[keys, q]  = kT_tile.T @ qT  (so attn@v needs no transpose)
  softmax without max-subtraction (scores are O(1); exp is safe in fp32);
  denominator comes free from an appended ones-column in v ("v_aug"),
  divide folded into the PSUM->SBUF copyback on DVE.
"""

import os
import sys

import ml_dtypes
import numpy as np

_bf16 = ml_dtypes.bfloat16

if "/opt/trn_rl_repo" not in sys.path:
    sys.path.insert(0, "/opt/trn_rl_repo")

B = 4
N = 2048
DIM = 768
HEADS = 12
DHEAD = 64
SCALE = DIM ** (-0.5)
NCORES = 8
HPC = 6  # heads per core
FEAT = HPC * DHEAD  # 384 per-core attention features

_PROGRAM = None  # (nc,) cached compiled bass program


def _build_program():
    from contextlib import ExitStack

    import concourse.bass as bass
    import concourse.tile as tile
    from concourse import bacc, mybir
    from concourse.masks import make_identity

    f32 = mybir.dt.float32
    f32r = mybir.dt.float32r
    bf16 = mybir.dt.bfloat16
    Alu = mybir.AluOpType
    ActF = mybir.ActivationFunctionType

    nc = bacc.Bacc("TRN2", target_bir_lowering=False, debug=False)

    x_in = nc.dram_tensor("x", [N, DIM], bf16, kind="ExternalInput")
    wqkvT = nc.dram_tensor("wqkvT", [DIM, 3 * FEAT], bf16, kind="ExternalInput")
    bqkv = nc.dram_tensor("bqkv", [128, 9], f32, kind="ExternalInput")
    woutT = nc.dram_tensor("woutT", [FEAT, DIM], f32r, kind="ExternalInput")
    out_T = nc.dram_tensor("outT", [DIM, N], f32, kind="ExternalOutput")

    NT = N // 128  # 16 n-tiles
    KC = DIM // 128  # 6 contraction chunks for dim
    NSPAN = N // 512  # 4 moving spans

    with tile.TileContext(nc) as tc, ExitStack() as ctx:
        const = ctx.enter_context(tc.tile_pool(name="const", bufs=1))
        identity_bf = const.tile([128, 128], bf16)
        make_identity(nc, identity_bf)
        ones_f32 = const.tile([128, 1], f32)
        nc.vector.memset(ones_f32[:, :], 1.0)
        ones65 = const.tile([65, 64], f32r)
        nc.vector.tensor_copy(
            out=ones65[:, :], in_=ones_f32[0:65, :].to_broadcast((65, 64))
        )
        bias_sb = const.tile([128, 9], f32)
        nc.gpsimd.dma_start(bias_sb[:, :], bqkv[:, :])

        # ---- Phase 1: x loads (bf16) + PE transposes into xT ----
        xt_pool = ctx.enter_context(tc.tile_pool(name="xT", bufs=1))
        xT = xt_pool.tile([128, KC, N], bf16)
        xin_pool = ctx.enter_context(tc.tile_pool(name="xin", bufs=4))
        xins = []
        for i in range(NT):
            xin = xin_pool.tile([128, DIM], bf16, name="xin", tag="xin", bufs=None)
            xins.append(xin)
        # x-tile DMAs first (first 8 gate the first qkv chains), then weights
        for i in range(8):
            eng = nc.gpsimd if i % 2 == 0 else nc.sync
            eng.dma_start(xins[i][:, :], x_in[i * 128 : (i + 1) * 128, :])

        wpool = ctx.enter_context(tc.tile_pool(name="w", bufs=1))
        w_all = wpool.tile([128, KC, 3 * FEAT], bf16)
        for j in range(KC):
            nc.gpsimd.dma_start(w_all[:, j, :], wqkvT[j * 128 : (j + 1) * 128, :])
        wout_sb = wpool.tile([128, 3, DIM], f32r)
        for c in range(3):
            nc.gpsimd.dma_start(wout_sb[:, c, :], woutT[c * 128 : (c + 1) * 128, :])
        for i in range(8, NT):
            eng = nc.gpsimd if i % 2 == 0 else nc.sync
            eng.dma_start(xins[i][:, :], x_in[i * 128 : (i + 1) * 128, :])

        # PSUM: spool 3x[128,2,512] (6 banks) + opool 2x[128,512] (2 banks)
        spool = ctx.enter_context(tc.tile_pool(name="spsum", bufs=3, space="PSUM"))
        opool = ctx.enter_context(tc.tile_pool(name="opsum", bufs=2, space="PSUM"))

        qk_pool = ctx.enter_context(tc.tile_pool(name="qk", bufs=2))
        vt_pool = ctx.enter_context(tc.tile_pool(name="vt", bufs=2))
        vaug_pool = ctx.enter_context(tc.tile_pool(name="vaug", bufs=2))
        exp_pool = ctx.enter_context(tc.tile_pool(name="expT", bufs=6))
        rcp_pool = ctx.enter_context(tc.tile_pool(name="rcp", bufs=2))
        sbo_pool = ctx.enter_context(tc.tile_pool(name="sbo", bufs=3))
        hst_pool = ctx.enter_context(tc.tile_pool(name="hstage", bufs=2))
        ao_pool = ctx.enter_context(tc.tile_pool(name="attnout", bufs=1))
        attn_outT = ao_pool.tile([128, 3, N], f32r)
        ost_pool = ctx.enter_context(tc.tile_pool(name="ostage", bufs=3))

        def emit_xtrans(i):
            for j in range(KC):
                tp = spool.tile([128, 128], bf16, tag="s", name="tp_x")
                nc.tensor.transpose(
                    tp[:, :], xins[i][:, j * 128 : (j + 1) * 128], identity_bf[:, :]
                )
                nc.vector.tensor_copy(
                    out=xT[:, j, i * 128 : (i + 1) * 128], in_=tp[:, :]
                )

        def emit_qkv_chain(qk_t, vT_t, hp, idx, m, sp2):
            """One (m, span-pair): 12 matmuls + biased copyback."""
            ps = spool.tile([128, 2, 512], f32, tag="s", name="ps_qkv")
            for u in range(2):
                span = 2 * sp2 + u
                for j in range(KC):
                    nc.tensor.matmul(
                        ps[:, u, :],
                        w_all[:, j, m * 128 : (m + 1) * 128],
                        xT[:, j, span * 512 : (span + 1) * 512],
                        start=(j == 0),
                        stop=(j == KC - 1),
                    )
            cols = slice(sp2 * 1024, (sp2 + 1) * 1024)
            if idx < 2:
                # head A -> chunk idx rows 0:64; head B -> chunk idx+2 rows 64:128
                nc.vector.tensor_scalar(
                    qk_t[0:64, idx, cols].rearrange("p (a b) -> p a b", a=2),
                    ps[0:64, :, :],
                    bias_sb[0:64, m : m + 1],
                    None,
                    Alu.add,
                )
                nc.vector.tensor_scalar(
                    qk_t[64:128, idx + 2, cols].rearrange("p (a b) -> p a b", a=2),
                    ps[64:128, :, :],
                    bias_sb[64:128, m : m + 1],
                    None,
                    Alu.add,
                )
            else:
                nc.vector.tensor_scalar(
                    vT_t[:, cols].rearrange("p (a b) -> p a b", a=2),
                    ps[:, :, :],
                    bias_sb[:, m : m + 1],
                    None,
                    Alu.add,
                )

        def emit_vtrans(vaug_t, vT_t, kc0, kcn):
            for kc in range(kc0, kcn):
                tp = spool.tile([128, 128], bf16, tag="s", name="tp_v")
                nc.tensor.transpose(
                    tp[:, :], vT_t[:, kc * 128 : (kc + 1) * 128], identity_bf[:, :]
                )
                nc.vector.tensor_copy(
                    out=vaug_t[:, kc, :].rearrange("p (t c) -> p t c", t=2)[
                        :, :, 0:64
                    ],
                    in_=tp[:, :].rearrange("p (t c) -> p t c", t=2),
                )


        def make_pair_units(hp):
            """Allocate tiles + return (tiles, list of PE filler closures)."""
            qk_t = qk_pool.tile([128, 4, N], bf16, name="qk", tag="qk")
            vT_t = vt_pool.tile([128, N], bf16, name="vT", tag="vT")
            vaug_t = vaug_pool.tile([128, NT, 256], bf16, name="vaug", tag="vaug")
            units = []

            def zero_pads():
                # zero the unused halves so K/M padding contributes nothing
                nc.gpsimd.memset(qk_t[64:128, 0:2, :], 0.0)
                nc.gpsimd.memset(qk_t[0:64, 2:4, :], 0.0)
                nc.gpsimd.memset(vaug_t[:, :, :], 0.0)

            units.append(zero_pads)
            order = [
                (0, hp, 0),
                (1, 3 + hp, 0),
                (2, 6 + hp, 0),
                (0, hp, 1),
                (1, 3 + hp, 1),
                (2, 6 + hp, 1),
            ]
            for idx, m, sp2 in order:
                units.append(
                    lambda i=idx, mm=m, s=sp2: emit_qkv_chain(qk_t, vT_t, hp, i, mm, s)
                )

            def vaug_init():
                ones_cols = vaug_t[:, :, :].rearrange("p k (t c) -> p k t c", t=2)[
                    :, :, :, 64:65
                ]
                nc.vector.tensor_copy(
                    out=ones_cols, in_=ones_f32[:, :].to_broadcast((128, NT, 2, 1))
                )
                emit_vtrans(vaug_t, vT_t, 0, 4)

            units.append(vaug_init)
            for kc0 in (4, 8, 12):
                units.append(lambda k=kc0: emit_vtrans(vaug_t, vT_t, k, k + 4))
            return (qk_t, vT_t, vaug_t), units

        def emit_outproj(m, span):
            """One out-proj tile [128, 512]: 3 matmuls + copyback + DMA."""
            ps = spool.tile([128, 2, 512], f32, tag="s", name="ps_op")
            for c in range(3):
                nc.tensor.matmul(
                    ps[:, 0, :],
                    wout_sb[:, c, m * 128 : (m + 1) * 128],
                    attn_outT[:, c, span * 512 : (span + 1) * 512],
                    start=(c == 0),
                    stop=(c == 2),
                )
            ostage = ost_pool.tile([128, 512], f32, name="ostage", tag="ostage")
            nc.vector.tensor_copy(out=ostage[:, :], in_=ps[:, 0, :])
            nc.gpsimd.dma_start(
                out_T[m * 128 : (m + 1) * 128, span * 512 : (span + 1) * 512],
                ostage[:, :],
            )

        # ---- phase 1 transposes + qkv/vaug for head pair 0, interleaved ----
        cur_tiles, units0 = make_pair_units(0)
        for i in range(8):
            emit_xtrans(i)
        for u in units0[0:3]:  # sp2=0 chains
            u()
        for i in range(8, NT):
            emit_xtrans(i)
        for u in units0[3:]:  # sp2=1 chains + vaug units
            u()

        # ---- attention per head pair, interleaving filler PE work ----
        for hp in range(3):
            qk, vT, vaug = cur_tiles
            if hp < 2:
                cur_tiles, filler = make_pair_units(hp + 1)
                fill_stride = max(1, (64 + len(filler)) // (len(filler) + 1))
            else:
                filler = []  # outproj units appended dynamically by normalize
                fill_stride = 1
            half_ctr = 0
            pending = [None]  # deferred normalize closure

            def flush_pending():
                if pending[0] is not None:
                    pending[0]()
                    pending[0] = None

            for j in range(2):
                qT = qk[:, 2 * j, :]
                kT = qk[:, 2 * j + 1, :]
                for span in range(NSPAN):
                    po = opool.tile([128, 512], f32, tag="o", name="po")
                    ets = []
                    for half in range(8):
                        ps = spool.tile([128, 2, 512], f32, tag="s", name="ps_s")
                        for u in range(2):
                            kc = 2 * half + u
                            nc.tensor.matmul(
                                ps[:, u, :],
                                kT[:, kc * 128 : (kc + 1) * 128],
                                qT[:, span * 512 : (span + 1) * 512],
                                start=True,
                                stop=True,
                            )
                        et = exp_pool.tile([128, 2, 512], bf16)
                        nc.scalar.activation(
                            et[:, :, :], ps[:, :, :], ActF.Exp, scale=float(SCALE)
                        )
                        ets.append(et)
                        if half == 1:
                            flush_pending()
                        if half >= 1:
                            pet = ets[half - 1]
                            for u in range(2):
                                kc = 2 * (half - 1) + u
                                nc.tensor.matmul(
                                    po[:, :],
                                    vaug[:, kc, j * 128 : (j + 1) * 128],
                                    pet[:, u, :],
                                    start=(kc == 0),
                                    stop=False,
                                )
                        half_ctr += 1
                        if hp == 2:
                            if len(filler) > 6:
                                filler.pop(0)()
                        elif filler and fill_stride and half_ctr % fill_stride == 0:
                            filler.pop(0)()
                    pet = ets[7]
                    for u in range(2):
                        kc = 14 + u
                        nc.tensor.matmul(
                            po[:, :],
                            vaug[:, kc, j * 128 : (j + 1) * 128],
                            pet[:, u, :],
                            start=False,
                            stop=(kc == 15),
                        )
                    # denominator recip + PSUM->SBUF copy issued immediately
                    # so the deferred broadcast matmul never waits on DVE
                    rs = rcp_pool.tile([65, 512], f32r, name="rs")
                    with nc.allow_low_precision(reason="fp32r recip"):
                        nc.vector.reciprocal(rs[64:65, :], po[64:65, :])
                    sb_o = sbo_pool.tile([65, 512], f32, name="sb_o")
                    nc.vector.tensor_copy(out=sb_o[:, :], in_=po[0:65, :])

                    def normalize(j=j, span=span, po=po, hp=hp, rs=rs, sb_o=sb_o):
                        # PE-broadcast of the recip back into po's own bank
                        # (WAR-ordered after the early copy), then divide.
                        nc.tensor.matmul(
                            po[0:64, :],
                            ones65[64:65, :],
                            rs[64:65, :],
                            start=True,
                            stop=True,
                        )
                        if j == 0:
                            ddst = attn_outT[0:64, hp, span * 512 : (span + 1) * 512]
                        else:
                            ddst = hst_pool.tile(
                                [64, 512], f32r, name="hstage", tag="hstage"
                            )
                        nc.vector.tensor_tensor(
                            out=ddst,
                            in0=sb_o[0:64, :],
                            in1=po[0:64, :],
                            op=Alu.mult,
                        )
                        if j == 1:
                            nc.gpsimd.dma_start(
                                attn_outT[64:128, hp, span * 512 : (span + 1) * 512],
                                ddst[:, :],
                            )
                            if hp == 2:
                                for m in range(DIM // 128):
                                    filler.append(
                                        lambda mm=m, s=span: emit_outproj(mm, s)
                                    )

                    pending[0] = normalize
            flush_pending()
            while filler:
                filler.pop(0)()

    nc.compile()
    return nc


def _get_program():
    global _PROGRAM
    if _PROGRAM is None:
        _PROGRAM = _build_program()
    return _PROGRAM


def _round_to_f32r(a):
    """Round fp32 to the PE's fp32r format: 11-bit mantissa, low 12 bits zero
    (round to nearest, ties away handled approximately via +0x7FF + lsb)."""
    u = np.ascontiguousarray(a, dtype=np.float32).view(np.uint32)
    r = u + np.uint32(0x7FF) + ((u >> np.uint32(12)) & np.uint32(1))
    r &= np.uint32(0xFFFFF000)
    return r.view(np.float32)


def make_core_inputs(x, w_qkv, b_qkv, w_out):
    """Host-side shard: per-core input dicts for cores 0..7."""
    x = np.asarray(x, dtype=np.float32)
    w_qkv = np.asarray(w_qkv, dtype=np.float32)
    b_qkv = np.asarray(b_qkv, dtype=np.float32)
    w_out = np.asarray(w_out, dtype=np.float32)

    per_group = []
    for g in range(2):
        rows = np.concatenate(
            [
                w_qkv[qkv * DIM + g * FEAT : qkv * DIM + (g + 1) * FEAT]
                for qkv in range(3)
            ],
            axis=0,
        )  # [1152, 768]
        wqkvT_g = np.ascontiguousarray(rows.T).astype(_bf16)  # [768, 1152]
        b_rows = np.concatenate(
            [
                b_qkv[qkv * DIM + g * FEAT : qkv * DIM + (g + 1) * FEAT]
                for qkv in range(3)
            ],
            axis=0,
        )  # [1152]
        bias_g = np.ascontiguousarray(b_rows.reshape(9, 128).T)  # [128, 9]
        woutT_g = _round_to_f32r(w_out[:, g * FEAT : (g + 1) * FEAT].T)
        per_group.append((wqkvT_g, bias_g, woutT_g))

    x_bf = [np.ascontiguousarray(x[b]).astype(_bf16) for b in range(B)]
    in_maps = []
    for c in range(NCORES):
        b, g = c // 2, c % 2
        wqkvT_g, bias_g, woutT_g = per_group[g]
        in_maps.append(
            {
                "x": x_bf[b],
                "wqkvT": wqkvT_g,
                "bqkv": bias_g,
                "woutT": woutT_g,
            }
        )
    return in_maps


def assemble_output(results, b_out):
    """Host-side unshard: sum partials per batch pair, transpose, add bias."""
    b_out = np.asarray(b_out, dtype=np.float32)
    out = np.empty((B, N, DIM), dtype=np.float32)
    for b in range(B):
        pT = results[2 * b]["outT"] + results[2 * b + 1]["outT"]  # [768, 2048]
        out[b] = pT.T + b_out[None, :]
    return out


def kernel(x, w_qkv, b_qkv, w_out, b_out):
    from concourse.bass_utils import run_bass_kernel_spmd

    nc = _get_program()
    in_maps = make_core_inputs(x, w_qkv, b_qkv, w_out)
    res = run_bass_kernel_spmd(nc, in_maps, list(range(NCORES)))
    return assemble_output(res.results, b_out)


# revision 33
# speedup vs baseline: 1.3167x; 1.0563x over previous
"""Multi-head attention forward on 8 Trainium2 NeuronCores (Bass/Tile).

Problem: x[4, 2048, 768] -> qkv proj (w_qkv[2304, 768]) -> 12-head attention
(softmax((q k^T) * 768^-0.5)) -> out proj (w_out[768, 768]).

Sharding: core c handles batch b = c//2 and a group of 6 heads g = c%2
(tensor parallel over heads within a batch pair). Each core computes a
partial output (its heads' contribution through the row-sliced out
projection, transposed: [768, 2048]); the host sums the two partials per
batch, transposes back and adds b_out.

Device-side layout notes (everything transposed so the contraction dim sits
on SBUF partitions):
  xT   [768, 2048]  built on-chip via PE transposes of x tiles
  qkvT [feat, 2048] = wT.T @ xT via fp32r matmuls (full-rate fp32)
  scoresT[keys, q]  = kT_tile.T @ qT  (so attn@v needs no transpose)
  softmax without max-subtraction (scores are O(1); exp is safe in fp32);
  denominator comes free from an appended ones-column in v ("v_aug"),
  divide folded into the PSUM->SBUF copyback on DVE.
"""

import os
import sys

import ml_dtypes
import numpy as np

_bf16 = ml_dtypes.bfloat16

if "/opt/trn_rl_repo" not in sys.path:
    sys.path.insert(0, "/opt/trn_rl_repo")

B = 4
N = 2048
DIM = 768
HEADS = 12
DHEAD = 64
SCALE = DIM ** (-0.5)
NCORES = 8
HPC = 6  # heads per core
FEAT = HPC * DHEAD  # 384 per-core attention features

_PROGRAM = None  # (nc,) cached compiled bass program


def _build_program():
    from contextlib import ExitStack

    import concourse.bass as bass
    import concourse.tile as tile
    from concourse import bacc, mybir
    from concourse.masks import make_identity

    f32 = mybir.dt.float32
    f32r = mybir.dt.float32r
    bf16 = mybir.dt.bfloat16
    Alu = mybir.AluOpType
    ActF = mybir.ActivationFunctionType

    nc = bacc.Bacc("TRN2", target_bir_lowering=False, debug=False)

    xT_in = nc.dram_tensor("xT", [DIM, N], bf16, kind="ExternalInput")
    wqkvT = nc.dram_tensor("wqkvT", [DIM, 3 * FEAT], bf16, kind="ExternalInput")
    bqkv = nc.dram_tensor("bqkv", [128, 9], f32, kind="ExternalInput")
    woutT = nc.dram_tensor("woutT", [FEAT, DIM], f32r, kind="ExternalInput")
    out_T = nc.dram_tensor("outT", [DIM, N], f32, kind="ExternalOutput")

    NT = N // 128  # 16 n-tiles
    KC = DIM // 128  # 6 contraction chunks for dim
    NSPAN = N // 512  # 4 moving spans

    with tile.TileContext(nc) as tc, ExitStack() as ctx:
        const = ctx.enter_context(tc.tile_pool(name="const", bufs=1))
        identity_bf = const.tile([128, 128], bf16)
        make_identity(nc, identity_bf)
        ones_f32 = const.tile([128, 1], f32)
        nc.vector.memset(ones_f32[:, :], 1.0)
        ones65 = const.tile([65, 64], f32r)
        nc.vector.tensor_copy(
            out=ones65[:, :], in_=ones_f32[0:65, :].to_broadcast((65, 64))
        )
        bias_sb = const.tile([128, 9], f32)
        nc.gpsimd.dma_start(bias_sb[:, :], bqkv[:, :])

        # ---- Phase 1: xT loads directly (host pre-transposed) + weights ----
        xt_pool = ctx.enter_context(tc.tile_pool(name="xT", bufs=1))
        xT = xt_pool.tile([128, KC, N], bf16)
        wpool = ctx.enter_context(tc.tile_pool(name="w", bufs=1))
        w_all = wpool.tile([128, KC, 3 * FEAT], bf16)
        # interleave x/w per contraction chunk so the first chains unblock fast
        for j in range(KC):
            nc.gpsimd.dma_start(
                xT[:, j, 0:1024], xT_in[j * 128 : (j + 1) * 128, 0:1024]
            )
            nc.sync.dma_start(w_all[:, j, :], wqkvT[j * 128 : (j + 1) * 128, :])
        for j in range(KC):
            eng = nc.gpsimd if j % 2 == 0 else nc.sync
            eng.dma_start(
                xT[:, j, 1024:2048], xT_in[j * 128 : (j + 1) * 128, 1024:2048]
            )
        wout_sb = wpool.tile([128, 3, DIM], f32r)
        for c in range(3):
            nc.gpsimd.dma_start(wout_sb[:, c, :], woutT[c * 128 : (c + 1) * 128, :])

        # PSUM: spool 3x[128,2,512] (6 banks) + opool 2x[128,512] (2 banks)
        spool = ctx.enter_context(tc.tile_pool(name="spsum", bufs=3, space="PSUM"))
        opool = ctx.enter_context(tc.tile_pool(name="opsum", bufs=2, space="PSUM"))

        qk_pool = ctx.enter_context(tc.tile_pool(name="qk", bufs=2))
        vt_pool = ctx.enter_context(tc.tile_pool(name="vt", bufs=2))
        vaug_pool = ctx.enter_context(tc.tile_pool(name="vaug", bufs=2))
        exp_pool = ctx.enter_context(tc.tile_pool(name="expT", bufs=6))
        rcp_pool = ctx.enter_context(tc.tile_pool(name="rcp", bufs=2))
        sbo_pool = ctx.enter_context(tc.tile_pool(name="sbo", bufs=3))
        hst_pool = ctx.enter_context(tc.tile_pool(name="hstage", bufs=2))
        ao_pool = ctx.enter_context(tc.tile_pool(name="attnout", bufs=1))
        attn_outT = ao_pool.tile([128, 3, N], f32r)
        ost_pool = ctx.enter_context(tc.tile_pool(name="ostage", bufs=3))

        def emit_qkv_chain(qk_t, vT_t, hp, idx, m, sp2):
            """One (m, span-pair): 12 matmuls + biased copyback."""
            ps = spool.tile([128, 2, 512], f32, tag="s", name="ps_qkv")
            for u in range(2):
                span = 2 * sp2 + u
                for j in range(KC):
                    nc.tensor.matmul(
                        ps[:, u, :],
                        w_all[:, j, m * 128 : (m + 1) * 128],
                        xT[:, j, span * 512 : (span + 1) * 512],
                        start=(j == 0),
                        stop=(j == KC - 1),
                    )
            cols = slice(sp2 * 1024, (sp2 + 1) * 1024)
            if idx < 2:
                # head A -> chunk idx rows 0:64; head B -> chunk idx+2 rows 64:128
                nc.vector.tensor_scalar(
                    qk_t[0:64, idx, cols].rearrange("p (a b) -> p a b", a=2),
                    ps[0:64, :, :],
                    bias_sb[0:64, m : m + 1],
                    None,
                    Alu.add,
                )
                nc.vector.tensor_scalar(
                    qk_t[64:128, idx + 2, cols].rearrange("p (a b) -> p a b", a=2),
                    ps[64:128, :, :],
                    bias_sb[64:128, m : m + 1],
                    None,
                    Alu.add,
                )
            else:
                nc.vector.tensor_scalar(
                    vT_t[:, cols].rearrange("p (a b) -> p a b", a=2),
                    ps[:, :, :],
                    bias_sb[:, m : m + 1],
                    None,
                    Alu.add,
                )

        def emit_vtrans(vaug_t, vT_t, kc0, kcn):
            for kc in range(kc0, kcn):
                tp = spool.tile([128, 128], bf16, tag="s", name="tp_v")
                nc.tensor.transpose(
                    tp[:, :], vT_t[:, kc * 128 : (kc + 1) * 128], identity_bf[:, :]
                )
                nc.vector.tensor_copy(
                    out=vaug_t[:, kc, :].rearrange("p (t c) -> p t c", t=2)[
                        :, :, 0:64
                    ],
                    in_=tp[:, :].rearrange("p (t c) -> p t c", t=2),
                )


        def make_pair_units(hp):
            """Allocate tiles + return (tiles, list of PE filler closures)."""
            qk_t = qk_pool.tile([128, 4, N], bf16, name="qk", tag="qk")
            vT_t = vt_pool.tile([128, N], bf16, name="vT", tag="vT")
            vaug_t = vaug_pool.tile([128, NT, 256], bf16, name="vaug", tag="vaug")
            units = []

            def zero_pads():
                # zero the unused halves so K/M padding contributes nothing
                nc.gpsimd.memset(qk_t[64:128, 0:2, :], 0.0)
                nc.gpsimd.memset(qk_t[0:64, 2:4, :], 0.0)
                nc.gpsimd.memset(vaug_t[:, :, :], 0.0)

            units.append(zero_pads)
            order = [
                (0, hp, 0),
                (1, 3 + hp, 0),
                (2, 6 + hp, 0),
                (0, hp, 1),
                (1, 3 + hp, 1),
                (2, 6 + hp, 1),
            ]
            for idx, m, sp2 in order:
                units.append(
                    lambda i=idx, mm=m, s=sp2: emit_qkv_chain(qk_t, vT_t, hp, i, mm, s)
                )

            def vaug_init():
                ones_cols = vaug_t[:, :, :].rearrange("p k (t c) -> p k t c", t=2)[
                    :, :, :, 64:65
                ]
                nc.vector.tensor_copy(
                    out=ones_cols, in_=ones_f32[:, :].to_broadcast((128, NT, 2, 1))
                )
                emit_vtrans(vaug_t, vT_t, 0, 4)

            units.append(vaug_init)
            for kc0 in (4, 8, 12):
                units.append(lambda k=kc0: emit_vtrans(vaug_t, vT_t, k, k + 4))
            return (qk_t, vT_t, vaug_t), units

        def emit_outproj(m, span):
            """One out-proj tile [128, 512]: 3 matmuls + copyback + DMA."""
            ps = spool.tile([128, 2, 512], f32, tag="s", name="ps_op")
            for c in range(3):
                nc.tensor.matmul(
                    ps[:, 0, :],
                    wout_sb[:, c, m * 128 : (m + 1) * 128],
                    attn_outT[:, c, span * 512 : (span + 1) * 512],
                    start=(c == 0),
                    stop=(c == 2),
                )
            ostage = ost_pool.tile([128, 512], f32, name="ostage", tag="ostage")
            nc.vector.tensor_copy(out=ostage[:, :], in_=ps[:, 0, :])
            nc.gpsimd.dma_start(
                out_T[m * 128 : (m + 1) * 128, span * 512 : (span + 1) * 512],
                ostage[:, :],
            )

        # ---- phase 1: qkv/vaug for head pair 0 (DMA-gated) ----
        cur_tiles, units0 = make_pair_units(0)
        for u in units0:
            u()

        # ---- attention per head pair, interleaving filler PE work ----
        for hp in range(3):
            qk, vT, vaug = cur_tiles
            if hp < 2:
                cur_tiles, filler = make_pair_units(hp + 1)
                fill_stride = max(1, (64 + len(filler)) // (len(filler) + 1))
            else:
                filler = []  # outproj units appended dynamically by normalize
                fill_stride = 1
            half_ctr = 0
            pending = [None]  # deferred normalize closure

            def flush_pending():
                if pending[0] is not None:
                    pending[0]()
                    pending[0] = None

            for j in range(2):
                qT = qk[:, 2 * j, :]
                kT = qk[:, 2 * j + 1, :]
                for span in range(NSPAN):
                    po = opool.tile([128, 512], f32, tag="o", name="po")
                    ets = []
                    for half in range(8):
                        ps = spool.tile([128, 2, 512], f32, tag="s", name="ps_s")
                        for u in range(2):
                            kc = 2 * half + u
                            nc.tensor.matmul(
                                ps[:, u, :],
                                kT[:, kc * 128 : (kc + 1) * 128],
                                qT[:, span * 512 : (span + 1) * 512],
                                start=True,
                                stop=True,
                            )
                        et = exp_pool.tile([128, 2, 512], bf16)
                        nc.scalar.activation(
                            et[:, :, :], ps[:, :, :], ActF.Exp, scale=float(SCALE)
                        )
                        ets.append(et)
                        if half == 1:
                            flush_pending()
                        if half >= 1:
                            pet = ets[half - 1]
                            for u in range(2):
                                kc = 2 * (half - 1) + u
                                nc.tensor.matmul(
                                    po[:, :],
                                    vaug[:, kc, j * 128 : (j + 1) * 128],
                                    pet[:, u, :],
                                    start=(kc == 0),
                                    stop=False,
                                )
                        half_ctr += 1
                        if hp == 2:
                            if len(filler) > 6:
                                filler.pop(0)()
                        elif filler and fill_stride and half_ctr % fill_stride == 0:
                            filler.pop(0)()
                    pet = ets[7]
                    for u in range(2):
                        kc = 14 + u
                        nc.tensor.matmul(
                            po[:, :],
                            vaug[:, kc, j * 128 : (j + 1) * 128],
                            pet[:, u, :],
                            start=False,
                            stop=(kc == 15),
                        )
                    # denominator recip + PSUM->SBUF copy issued immediately
                    # so the deferred broadcast matmul never waits on DVE
                    rs = rcp_pool.tile([65, 512], f32r, name="rs")
                    with nc.allow_low_precision(reason="fp32r recip"):
                        nc.vector.reciprocal(rs[64:65, :], po[64:65, :])
                    sb_o = sbo_pool.tile([65, 512], f32, name="sb_o")
                    nc.vector.tensor_copy(out=sb_o[:, :], in_=po[0:65, :])

                    def normalize(j=j, span=span, po=po, hp=hp, rs=rs, sb_o=sb_o):
                        # PE-broadcast of the recip back into po's own bank
                        # (WAR-ordered after the early copy), then divide.
                        nc.tensor.matmul(
                            po[0:64, :],
                            ones65[64:65, :],
                            rs[64:65, :],
                            start=True,
                            stop=True,
                        )
                        if j == 0:
                            ddst = attn_outT[0:64, hp, span * 512 : (span + 1) * 512]
                        else:
                            ddst = hst_pool.tile(
                                [64, 512], f32r, name="hstage", tag="hstage"
                            )
                        nc.vector.tensor_tensor(
                            out=ddst,
                            in0=sb_o[0:64, :],
                            in1=po[0:64, :],
                            op=Alu.mult,
                        )
                        if j == 1:
                            nc.gpsimd.dma_start(
                                attn_outT[64:128, hp, span * 512 : (span + 1) * 512],
                                ddst[:, :],
                            )
                            if hp == 2:
                                for m in range(DIM // 128):
                                    filler.append(
                                        lambda mm=m, s=span: emit_outproj(mm, s)
                                    )

                    pending[0] = normalize
            flush_pending()
            while filler:
                filler.pop(0)()

    nc.compile()
    return nc


def _get_program():
    global _PROGRAM
    if _PROGRAM is None:
        _PROGRAM = _build_program()
    return _PROGRAM


def _round_to_f32r(a):
    """Round fp32 to the PE's fp32r format: 11-bit mantissa, low 12 bits zero
    (round to nearest, ties away handled approximately via +0x7FF + lsb)."""
    u = np.ascontiguousarray(a, dtype=np.float32).view(np.uint32)
    r = u + np.uint32(0x7FF) + ((u >> np.uint32(12)) & np.uint32(1))
    r &= np.uint32(0xFFFFF000)
    return r.view(np.float32)


def make_core_inputs(x, w_qkv, b_qkv, w_out):
    """Host-side shard: per-core input dicts for cores 0..7."""
    x = np.asarray(x, dtype=np.float32)
    w_qkv = np.asarray(w_qkv, dtype=np.float32)
    b_qkv = np.asarray(b_qkv, dtype=np.float32)
    w_out = np.asarray(w_out, dtype=np.float32)

    per_group = []
    for g in range(2):
        rows = np.concatenate(
            [
                w_qkv[qkv * DIM + g * FEAT : qkv * DIM + (g + 1) * FEAT]
                for qkv in range(3)
            ],
            axis=0,
        )  # [1152, 768]
        wqkvT_g = np.ascontiguousarray(rows.T).astype(_bf16)  # [768, 1152]
        b_rows = np.concatenate(
            [
                b_qkv[qkv * DIM + g * FEAT : qkv * DIM + (g + 1) * FEAT]
                for qkv in range(3)
            ],
            axis=0,
        )  # [1152]
        bias_g = np.ascontiguousarray(b_rows.reshape(9, 128).T)  # [128, 9]
        woutT_g = _round_to_f32r(w_out[:, g * FEAT : (g + 1) * FEAT].T)
        per_group.append((wqkvT_g, bias_g, woutT_g))

    xT_bf = [np.ascontiguousarray(x[b].T).astype(_bf16) for b in range(B)]
    in_maps = []
    for c in range(NCORES):
        b, g = c // 2, c % 2
        wqkvT_g, bias_g, woutT_g = per_group[g]
        in_maps.append(
            {
                "xT": xT_bf[b],
                "wqkvT": wqkvT_g,
                "bqkv": bias_g,
                "woutT": woutT_g,
            }
        )
    return in_maps


def assemble_output(results, b_out):
    """Host-side unshard: sum partials per batch pair, transpose, add bias."""
    b_out = np.asarray(b_out, dtype=np.float32)
    out = np.empty((B, N, DIM), dtype=np.float32)
    for b in range(B):
        pT = results[2 * b]["outT"] + results[2 * b + 1]["outT"]  # [768, 2048]
        out[b] = pT.T + b_out[None, :]
    return out


def kernel(x, w_qkv, b_qkv, w_out, b_out):
    from concourse.bass_utils import run_bass_kernel_spmd

    nc = _get_program()
    in_maps = make_core_inputs(x, w_qkv, b_qkv, w_out)
    res = run_bass_kernel_spmd(nc, in_maps, list(range(NCORES)))
    return assemble_output(res.results, b_out)


# revision 37
# speedup vs baseline: 1.3681x; 1.0390x over previous
"""Multi-head attention forward on 8 Trainium2 NeuronCores (Bass/Tile).

Problem: x[4, 2048, 768] -> qkv proj (w_qkv[2304, 768]) -> 12-head attention
(softmax((q k^T) * 768^-0.5)) -> out proj (w_out[768, 768]).

Sharding: core c handles batch b = c//2 and a group of 6 heads g = c%2
(tensor parallel over heads within a batch pair). Each core computes a
partial output (its heads' contribution through the row-sliced out
projection, transposed: [768, 2048]); the host sums the two partials per
batch, transposes back and adds b_out.

Device-side layout notes (everything transposed so the contraction dim sits
on SBUF partitions):
  xT   [768, 2048]  built on-chip via PE transposes of x tiles
  qkvT [feat, 2048] = wT.T @ xT via fp32r matmuls (full-rate fp32)
  scoresT[keys, q]  = kT_tile.T @ qT  (so attn@v needs no transpose)
  softmax without max-subtraction (scores are O(1); exp is safe in fp32);
  denominator comes free from an appended ones-column in v ("v_aug"),
  divide folded into the PSUM->SBUF copyback on DVE.
"""

import os
import sys

import ml_dtypes
import numpy as np

_bf16 = ml_dtypes.bfloat16

if "/opt/trn_rl_repo" not in sys.path:
    sys.path.insert(0, "/opt/trn_rl_repo")

B = 4
N = 2048
DIM = 768
HEADS = 12
DHEAD = 64
SCALE = DIM ** (-0.5)
NCORES = 8
HPC = 6  # heads per core
FEAT = HPC * DHEAD  # 384 per-core attention features

_PROGRAM = None  # (nc,) cached compiled bass program


def _build_program():
    from contextlib import ExitStack

    import concourse.bass as bass
    import concourse.tile as tile
    from concourse import bacc, mybir
    from concourse.masks import make_identity

    f32 = mybir.dt.float32
    f32r = mybir.dt.float32r
    bf16 = mybir.dt.bfloat16
    Alu = mybir.AluOpType
    ActF = mybir.ActivationFunctionType

    nc = bacc.Bacc("TRN2", target_bir_lowering=False, debug=False)

    xT_in = nc.dram_tensor("xT", [DIM, N], bf16, kind="ExternalInput")
    wqkvT = nc.dram_tensor("wqkvT", [DIM, 3 * FEAT], bf16, kind="ExternalInput")
    bqkv = nc.dram_tensor("bqkv", [128, 9], f32, kind="ExternalInput")
    woutT = nc.dram_tensor("woutT", [FEAT, DIM], f32r, kind="ExternalInput")
    out_T = nc.dram_tensor("outT", [DIM, N], f32, kind="ExternalOutput")

    NT = N // 128  # 16 n-tiles
    KC = DIM // 128  # 6 contraction chunks for dim
    NSPAN = N // 512  # 4 moving spans

    with tile.TileContext(nc) as tc, ExitStack() as ctx:
        const = ctx.enter_context(tc.tile_pool(name="const", bufs=1))
        identity_bf = const.tile([128, 128], bf16)
        make_identity(nc, identity_bf)
        ones_f32 = const.tile([128, 1], f32)
        nc.vector.memset(ones_f32[:, :], 1.0)
        ones65 = const.tile([65, 64], f32r)
        nc.vector.tensor_copy(
            out=ones65[:, :], in_=ones_f32[0:65, :].to_broadcast((65, 64))
        )
        bias_sb = const.tile([128, 9], f32)
        nc.gpsimd.dma_start(bias_sb[:, :], bqkv[:, :])

        # ---- Phase 1: xT loads directly (host pre-transposed) + weights ----
        xt_pool = ctx.enter_context(tc.tile_pool(name="xT", bufs=1))
        xT = xt_pool.tile([128, KC, N], bf16)
        wpool = ctx.enter_context(tc.tile_pool(name="w", bufs=1))
        w_all = wpool.tile([128, KC, 3 * FEAT], bf16)
        # interleave x/w per contraction chunk so the first chains unblock fast
        for j in range(KC):
            nc.gpsimd.dma_start(
                xT[:, j, 0:1024], xT_in[j * 128 : (j + 1) * 128, 0:1024]
            )
            nc.sync.dma_start(w_all[:, j, :], wqkvT[j * 128 : (j + 1) * 128, :])
        for j in range(KC):
            eng = nc.gpsimd if j % 2 == 0 else nc.sync
            eng.dma_start(
                xT[:, j, 1024:2048], xT_in[j * 128 : (j + 1) * 128, 1024:2048]
            )
        wout_sb = wpool.tile([128, 3, DIM], f32r)
        for c in range(3):
            nc.gpsimd.dma_start(wout_sb[:, c, :], woutT[c * 128 : (c + 1) * 128, :])

        # PSUM: spool 3x[128,2,512] (6 banks) + opool 2x[128,512] (2 banks)
        spool = ctx.enter_context(tc.tile_pool(name="spsum", bufs=3, space="PSUM"))
        opool = ctx.enter_context(tc.tile_pool(name="opsum", bufs=2, space="PSUM"))

        qk_pool = ctx.enter_context(tc.tile_pool(name="qk", bufs=2))
        vt_pool = ctx.enter_context(tc.tile_pool(name="vt", bufs=2))
        vaug_pool = ctx.enter_context(tc.tile_pool(name="vaug", bufs=2))
        exp_pool = ctx.enter_context(tc.tile_pool(name="expT", bufs=6))
        rcp_pool = ctx.enter_context(tc.tile_pool(name="rcp", bufs=2))
        sbo_pool = ctx.enter_context(tc.tile_pool(name="sbo", bufs=3))
        hst_pool = ctx.enter_context(tc.tile_pool(name="hstage", bufs=2))
        ao_pool = ctx.enter_context(tc.tile_pool(name="attnout", bufs=1))
        attn_outT = ao_pool.tile([128, 3, N], f32r)
        ost_pool = ctx.enter_context(tc.tile_pool(name="ostage", bufs=3))

        def emit_qkv_chain(qk_t, vT_t, hp, idx, m, sp2):
            """One (m, span-pair): 12 matmuls + biased copyback."""
            ps = spool.tile([128, 2, 512], f32, tag="s", name="ps_qkv")
            for u in range(2):
                span = 2 * sp2 + u
                for j in range(KC):
                    nc.tensor.matmul(
                        ps[:, u, :],
                        w_all[:, j, m * 128 : (m + 1) * 128],
                        xT[:, j, span * 512 : (span + 1) * 512],
                        start=(j == 0),
                        stop=(j == KC - 1),
                    )
            cols = slice(sp2 * 1024, (sp2 + 1) * 1024)
            if idx < 2:
                # head A -> chunk idx rows 0:64; head B -> chunk idx+2 rows 64:128
                nc.vector.tensor_scalar(
                    qk_t[0:64, idx, cols].rearrange("p (a b) -> p a b", a=2),
                    ps[0:64, :, :],
                    bias_sb[0:64, m : m + 1],
                    None,
                    Alu.add,
                )
                nc.vector.tensor_scalar(
                    qk_t[64:128, idx + 2, cols].rearrange("p (a b) -> p a b", a=2),
                    ps[64:128, :, :],
                    bias_sb[64:128, m : m + 1],
                    None,
                    Alu.add,
                )
            else:
                nc.vector.tensor_scalar(
                    vT_t[:, cols].rearrange("p (a b) -> p a b", a=2),
                    ps[:, :, :],
                    bias_sb[:, m : m + 1],
                    None,
                    Alu.add,
                )

        def emit_vtrans(vaug_t, vT_t, kc0, kcn):
            for kc in range(kc0, kcn):
                tp = spool.tile([128, 128], bf16, tag="s", name="tp_v")
                nc.tensor.transpose(
                    tp[:, :], vT_t[:, kc * 128 : (kc + 1) * 128], identity_bf[:, :]
                )
                nc.vector.tensor_copy(
                    out=vaug_t[:, kc, :].rearrange("p (t c) -> p t c", t=2)[
                        :, :, 0:64
                    ],
                    in_=tp[:, :].rearrange("p (t c) -> p t c", t=2),
                )


        def make_pair_units(hp):
            """Allocate tiles + return (tiles, list of PE filler closures)."""
            qk_t = qk_pool.tile([128, 4, N], bf16, name="qk", tag="qk")
            vT_t = vt_pool.tile([128, N], bf16, name="vT", tag="vT")
            vaug_t = vaug_pool.tile([128, NT, 256], bf16, name="vaug", tag="vaug")
            units = []

            def zero_pads():
                # zero the unused halves so K/M padding contributes nothing
                nc.gpsimd.memset(qk_t[64:128, 0:2, :], 0.0)
                nc.gpsimd.memset(qk_t[0:64, 2:4, :], 0.0)
                nc.gpsimd.memset(vaug_t[:, :, :], 0.0)

            units.append(zero_pads)
            order = [
                (0, hp, 0),
                (1, 3 + hp, 0),
                (2, 6 + hp, 0),
                (0, hp, 1),
                (1, 3 + hp, 1),
                (2, 6 + hp, 1),
            ]
            for idx, m, sp2 in order:
                units.append(
                    lambda i=idx, mm=m, s=sp2: emit_qkv_chain(qk_t, vT_t, hp, i, mm, s)
                )

            def vaug_init():
                ones_cols = vaug_t[:, :, :].rearrange("p k (t c) -> p k t c", t=2)[
                    :, :, :, 64:65
                ]
                nc.vector.tensor_copy(
                    out=ones_cols, in_=ones_f32[:, :].to_broadcast((128, NT, 2, 1))
                )
                emit_vtrans(vaug_t, vT_t, 0, 4)

            units.append(vaug_init)
            for kc0 in (4, 8, 12):
                units.append(lambda k=kc0: emit_vtrans(vaug_t, vT_t, k, k + 4))
            return (qk_t, vT_t, vaug_t), units

        def emit_outproj(m, span):
            """One out-proj tile [128, 512]: 3 matmuls + copyback + DMA."""
            ps = spool.tile([128, 2, 512], f32, tag="s", name="ps_op")
            for c in range(3):
                nc.tensor.matmul(
                    ps[:, 0, :],
                    wout_sb[:, c, m * 128 : (m + 1) * 128],
                    attn_outT[:, c, span * 512 : (span + 1) * 512],
                    start=(c == 0),
                    stop=(c == 2),
                )
            ostage = ost_pool.tile([128, 512], f32, name="ostage", tag="ostage")
            nc.vector.tensor_copy(out=ostage[:, :], in_=ps[:, 0, :])
            nc.gpsimd.dma_start(
                out_T[m * 128 : (m + 1) * 128, span * 512 : (span + 1) * 512],
                ostage[:, :],
            )

        # ---- phase 1: qkv/vaug for head pair 0 (DMA-gated) ----
        cur_tiles, units0 = make_pair_units(0)
        for u in units0:
            u()

        # ---- attention per head pair, interleaving filler PE work ----
        for hp in range(3):
            qk, vT, vaug = cur_tiles
            if hp < 2:
                cur_tiles, filler = make_pair_units(hp + 1)
                fill_stride = max(1, (64 + len(filler)) // (len(filler) + 1))
            else:
                filler = []  # outproj units appended dynamically by normalize
                fill_stride = 1
            half_ctr = 0
            pending = [None]  # deferred normalize closure

            def flush_pending():
                if pending[0] is not None:
                    pending[0]()
                    pending[0] = None

            for j in range(2):
                qT = qk[:, 2 * j, :]
                kT = qk[:, 2 * j + 1, :]
                for span in range(NSPAN):
                    po = opool.tile([128, 512], f32, tag="o", name="po")
                    ets = []
                    for half in range(8):
                        ps = spool.tile([128, 2, 512], f32, tag="s", name="ps_s")
                        for u in range(2):
                            kc = 2 * half + u
                            nc.tensor.matmul(
                                ps[:, u, :],
                                kT[:, kc * 128 : (kc + 1) * 128],
                                qT[:, span * 512 : (span + 1) * 512],
                                start=True,
                                stop=True,
                            )
                        et = exp_pool.tile([128, 2, 512], bf16)
                        nc.scalar.activation(
                            et[:, :, :], ps[:, :, :], ActF.Exp, scale=float(SCALE)
                        )
                        ets.append(et)
                        if half == 5:
                            # deferred far enough that the previous block's
                            # 3.4us DVE reciprocal has finished — the PE
                            # broadcast below never stalls
                            flush_pending()
                        if half >= 1:
                            pet = ets[half - 1]
                            for u in range(2):
                                kc = 2 * (half - 1) + u
                                nc.tensor.matmul(
                                    po[:, :],
                                    vaug[:, kc, j * 128 : (j + 1) * 128],
                                    pet[:, u, :],
                                    start=(kc == 0),
                                    stop=False,
                                )
                        half_ctr += 1
                        if hp == 2:
                            if len(filler) > 6:
                                filler.pop(0)()
                        elif filler and fill_stride and half_ctr % fill_stride == 0:
                            filler.pop(0)()
                    pet = ets[7]
                    for u in range(2):
                        kc = 14 + u
                        nc.tensor.matmul(
                            po[:, :],
                            vaug[:, kc, j * 128 : (j + 1) * 128],
                            pet[:, u, :],
                            start=False,
                            stop=(kc == 15),
                        )
                    # denominator recip + PSUM->SBUF copy issued immediately
                    # so the deferred broadcast matmul never waits on DVE
                    rs = rcp_pool.tile([65, 512], f32r, name="rs")
                    with nc.allow_low_precision(reason="fp32r recip"):
                        nc.vector.reciprocal(rs[64:65, :], po[64:65, :])
                    sb_o = sbo_pool.tile([65, 512], f32, name="sb_o")
                    nc.vector.tensor_copy(out=sb_o[:, :], in_=po[0:65, :])

                    def normalize(j=j, span=span, po=po, hp=hp, rs=rs, sb_o=sb_o):
                        # PE-broadcast of the recip back into po's own bank
                        # (WAR-ordered after the early copy), then divide.
                        nc.tensor.matmul(
                            po[0:64, :],
                            ones65[64:65, :],
                            rs[64:65, :],
                            start=True,
                            stop=True,
                        )
                        if j == 0:
                            ddst = attn_outT[0:64, hp, span * 512 : (span + 1) * 512]
                        else:
                            ddst = hst_pool.tile(
                                [64, 512], f32r, name="hstage", tag="hstage"
                            )
                        nc.vector.tensor_tensor(
                            out=ddst,
                            in0=sb_o[0:64, :],
                            in1=po[0:64, :],
                            op=Alu.mult,
                        )
                        if j == 1:
                            nc.gpsimd.dma_start(
                                attn_outT[64:128, hp, span * 512 : (span + 1) * 512],
                                ddst[:, :],
                            )
                            if hp == 2:
                                for m in range(DIM // 128):
                                    filler.append(
                                        lambda mm=m, s=span: emit_outproj(mm, s)
                                    )

                    pending[0] = normalize
            flush_pending()
            while filler:
                filler.pop(0)()

    nc.compile()
    return nc


def _get_program():
    global _PROGRAM
    if _PROGRAM is None:
        _PROGRAM = _build_program()
    return _PROGRAM


def _round_to_f32r(a):
    """Round fp32 to the PE's fp32r format: 11-bit mantissa, low 12 bits zero
    (round to nearest, ties away handled approximately via +0x7FF + lsb)."""
    u = np.ascontiguousarray(a, dtype=np.float32).view(np.uint32)
    r = u + np.uint32(0x7FF) + ((u >> np.uint32(12)) & np.uint32(1))
    r &= np.uint32(0xFFFFF000)
    return r.view(np.float32)


def make_core_inputs(x, w_qkv, b_qkv, w_out):
    """Host-side shard: per-core input dicts for cores 0..7."""
    x = np.asarray(x, dtype=np.float32)
    w_qkv = np.asarray(w_qkv, dtype=np.float32)
    b_qkv = np.asarray(b_qkv, dtype=np.float32)
    w_out = np.asarray(w_out, dtype=np.float32)

    per_group = []
    for g in range(2):
        rows = np.concatenate(
            [
                w_qkv[qkv * DIM + g * FEAT : qkv * DIM + (g + 1) * FEAT]
                for qkv in range(3)
            ],
            axis=0,
        )  # [1152, 768]
        wqkvT_g = np.ascontiguousarray(rows.T).astype(_bf16)  # [768, 1152]
        b_rows = np.concatenate(
            [
                b_qkv[qkv * DIM + g * FEAT : qkv * DIM + (g + 1) * FEAT]
                for qkv in range(3)
            ],
            axis=0,
        )  # [1152]
        bias_g = np.ascontiguousarray(b_rows.reshape(9, 128).T)  # [128, 9]
        woutT_g = _round_to_f32r(w_out[:, g * FEAT : (g + 1) * FEAT].T)
        per_group.append((wqkvT_g, bias_g, woutT_g))

    xT_bf = [np.ascontiguousarray(x[b].T).astype(_bf16) for b in range(B)]
    in_maps = []
    for c in range(NCORES):
        b, g = c // 2, c % 2
        wqkvT_g, bias_g, woutT_g = per_group[g]
        in_maps.append(
            {
                "xT": xT_bf[b],
                "wqkvT": wqkvT_g,
                "bqkv": bias_g,
                "woutT": woutT_g,
            }
        )
    return in_maps


def assemble_output(results, b_out):
    """Host-side unshard: sum partials per batch pair, transpose, add bias."""
    b_out = np.asarray(b_out, dtype=np.float32)
    out = np.empty((B, N, DIM), dtype=np.float32)
    for b in range(B):
        pT = results[2 * b]["outT"] + results[2 * b + 1]["outT"]  # [768, 2048]
        out[b] = pT.T + b_out[None, :]
    return out


def kernel(x, w_qkv, b_qkv, w_out, b_out):
    from concourse.bass_utils import run_bass_kernel_spmd

    nc = _get_program()
    in_maps = make_core_inputs(x, w_qkv, b_qkv, w_out)
    res = run_bass_kernel_spmd(nc, in_maps, list(range(NCORES)))
    return assemble_output(res.results, b_out)
